# revision 2
# baseline (speedup 1.0000x reference)
"""Single-head causal attention on 8 Trainium2 NeuronCores (Bass/Tile).

Problem: B=4, S=2048, D=E=1024 fp32.
  K = Xk @ WK; V = Xv @ WV; Q = Xq @ WQ
  att = softmax(causal(Q K^T / sqrt(S))) @ V;  returns (Q, att)

Sharding (uniform SPMD program, per-core differences are data only):
  core c -> batch b = c // 2, shard s = c % 2.
  Core-local q-tile t in [0,8) covers absolute query rows
  [(2t+s)*128, (2t+s+1)*128), i.e. the batch's 16 query tiles of 128 rows
  are dealt alternately to the two cores of the pair.  q-tiles are packed
  per core into a [1024, 1024] "local" query space.  Tiles are processed
  in PAIRS u in [0,4) (local columns [256u, 256u+256)); pair u attends kv
  chunks [0, 4u+4) of 128 keys, padded to a shard-independent trip count;
  the causal boundary and padding are enforced by 4 host-supplied
  multiplicative masks (shift-invariant across pairs, shard-dependent).

Per-core kernel:
  - On-chip transposes of X (PE transpose via identity) give X^T with the
    contraction dim d on partitions.
  - Projections as fp32r matmuls (full PE rate at moving dim >= 256):
      Q  [q,e] : lhsT = Xq^T[d,q] chunk, rhs = WQ[d,e]
      K^T[e,k] : lhsT = WK[d,e] chunk,  rhs = Xk^T[d,k]
      V  [k,e] : lhsT = Xv^T[d,k] chunk, rhs = WV[d,e]
    Q^T for the score matmuls is a PE re-transpose of the Q tiles.
  - Scores computed TRANSPOSED, S^T[k,q] (avoids transposing P):
      lhsT = K^T[e,k] chunk, rhs = Q^T[e,q]
    P^T = exp(S^T * scale) (no max subtraction: |scores| <~ 2 here),
    causal/padding via mask multiply, denominator l via an all-ones
    rhs matmul, att rows = (P^T.T @ V) / l.
  - KV is processed in two passes of 1024 keys to halve K^T/V residency;
    pairs 2,3 carry partial (att, l) accumulators across the passes.
"""

import math
import sys

sys.path.insert(0, "/opt/trn_rl_repo")

import numpy as np  # noqa: E402

import concourse.bass as bass  # noqa: E402
import concourse.tile as tile  # noqa: E402
from concourse import bacc, mybir  # noqa: E402
from concourse.bass_utils import run_bass_kernel_spmd  # noqa: E402
from concourse.masks import make_identity  # noqa: E402

B, S, D, E = 4, 2048, 1024, 1024
QL = 1024  # per-core local query rows
NCORES = 8
SCALE = 1.0 / math.sqrt(float(S))
F32 = mybir.dt.float32
F32R = mybir.dt.float32r

KC = 128  # kv chunk (S^T partition tile)
PASS_KC = 8  # kv chunks per pass
NPASS = 2


def _dchunks(ap3, dc):
    return ap3[:, dc]


def build_nc(reps: int = 1, timing: bool = False, phase: str = 'full'):
    nc = bacc.Bacc("TRN2", target_bir_lowering=False, debug=False, num_devices=NCORES)

    xq_d = nc.dram_tensor("xq", [QL, D], F32R, kind="ExternalInput").ap()
    xk_d = nc.dram_tensor("xk", [S, D], F32R, kind="ExternalInput").ap()
    xv_d = nc.dram_tensor("xv", [S, D], F32R, kind="ExternalInput").ap()
    wq_d = nc.dram_tensor("wq", [D, E], F32R, kind="ExternalInput").ap()
    wk_d = nc.dram_tensor("wk", [D, E], F32R, kind="ExternalInput").ap()
    wv_d = nc.dram_tensor("wv", [D, E], F32R, kind="ExternalInput").ap()
    mk_d = nc.dram_tensor("masks", [4, 128, 256], F32R, kind="ExternalInput").ap()
    okind = "Internal" if timing else "ExternalOutput"
    qo_d = nc.dram_tensor("q_out", [QL, E], F32R, kind=okind).ap()
    ao_d = nc.dram_tensor("att_out", [QL, E], F32, kind=okind).ap()
    a0_d = nc.dram_tensor("a0_out", [512, E], F32, kind=okind).ap()
    l0_d = nc.dram_tensor("l0_out", [2, 2, 128], F32, kind=okind).ap()
    l1_d = nc.dram_tensor("l1_out", [2, 2, 128], F32, kind=okind).ap()
    done_d = (
        nc.dram_tensor("done", [1, 2], F32, kind="ExternalOutput").ap()
        if timing
        else None
    )

    with tile.TileContext(nc) as tc:
        _emit(tc, reps, xq_d, xk_d, xv_d, wq_d, wk_d, wv_d, mk_d, qo_d, ao_d,
              a0_d, l0_d, l1_d, done_d, phase)
    nc.compile()
    return nc


def _emit(tc, reps, xq_d, xk_d, xv_d, wq_d, wk_d, wv_d, mk_d, qo_d, ao_d,
          a0_d, l0_d, l1_d, done_d=None, phase="full"):
    nc = tc.nc
    with (
        tc.tile_pool(name="const", bufs=1) as cpool,
        tc.tile_pool(name="wp", bufs=8) as wpool,
        tc.tile_pool(name="xload", bufs=2) as xlpool,
        tc.tile_pool(name="xt", bufs=2) as xtpool,
        tc.tile_pool(name="big", bufs=1) as bigpool,
        tc.tile_pool(name="pt", bufs=2) as ptpool,
        tc.tile_pool(name="outp", bufs=2) as outpool,
        tc.tile_pool(name="smallp", bufs=2) as smallpool,
        tc.tile_pool(name="dram", bufs=2, space="DRAM") as drampool,
        tc.tile_pool(name="ps", bufs=2, space="PSUM") as pspool,
        tc.tile_pool(name="psa", bufs=2, space="PSUM") as psapool,
        tc.tile_pool(name="psl", bufs=2, space="PSUM") as pslpool,
    ):
        ident_f = cpool.tile([128, 128], F32)
        make_identity(nc, ident_f[:])
        ident = cpool.tile([128, 128], F32R)
        nc.vector.tensor_copy(ident[:], ident_f[:])
        ones_f = cpool.tile([128, 2], F32)
        nc.vector.memset(ones_f[:], 1.0)
        if done_d is not None:
            nc.sync.dma_start(done_d[:], ones_f[0:1, :])
        ones = cpool.tile([128, 2], F32R)
        nc.vector.tensor_copy(ones[:], ones_f[:])
        maskt = cpool.tile([128, 4, 256], F32R)

        def xt_strip(x_dram, row0, nrows, name):
            """Load X[row0:row0+nrows, :] and return X^T strip [128, 8, nrows]."""
            strip = xtpool.tile([128, D // 128, 256], F32R, tag="xts", name=name)
            for h in range(nrows // 128):
                xl = xlpool.tile([128, D], F32R, tag="xl", name=f"{name}_l{h}")
                nc.sync.dma_start(xl[:], x_dram[row0 + h * 128 : row0 + (h + 1) * 128, :])
                for dc in range(D // 128):
                    pst = pspool.tile([128, 256], F32R, tag="ps", name=f"{name}_t")
                    nc.tensor.transpose(
                        pst[:, :128], xl[:, dc * 128 : dc * 128 + 128], ident[:]
                    )
                    nc.vector.tensor_copy(
                        strip[:, dc, h * 128 : h * 128 + 128], pst[:, :128]
                    )
            return strip

        for _rep in range(reps):
            def wload(w_d, nm):
                qs = []
                for i in range(4):
                    t = wpool.tile([128, D // 128, 256], F32R, tag="w",
                                   name=f"{nm}{i}")
                    nc.sync.dma_start(
                        t[:],
                        w_d.rearrange("(c p) e -> p c e", p=128)[
                            :, :, i * 256 : i * 256 + 256
                        ],
                    )
                    qs.append(t)
                return qs

            qt_big = bigpool.tile([128, E // 128, QL], F32R, tag="qt_big", name="qt_big")
            def kv_proj_pass(p):
                """Project all of pass p's kv rows (no pair exchange)."""
                krow0 = p * PASS_KC * KC
                kt_big = bigpool.tile(
                    [128, E // 128, PASS_KC * KC], F32R, tag="kt", name=f"kt{p}"
                )
                v_big = bigpool.tile(
                    [128, PASS_KC, E], F32R, tag="v", name=f"v{p}"
                )
                nstr = PASS_KC * KC // 256
                kts = [xt_strip(xk_d, krow0, 256, f"xkt{p}_0")]
                vts = [xt_strip(xv_d, krow0, 256, f"xvt{p}_0")]
                for j in range(nstr):
                    xkt, xvt = kts[j], vts[j]
                    for ec in range(E // 128):
                        ps = pspool.tile([128, 256], F32, tag="ps", name="kps")
                        for dc in range(D // 128):
                            nc.tensor.matmul(
                                ps[:],
                                wkq[ec // 2][:, dc, (ec % 2) * 128 : (ec % 2) * 128 + 128],
                                xkt[:, dc, :],
                                start=(dc == 0),
                                stop=(dc == D // 128 - 1),
                            )
                        if ec % 2 == 0:
                            nc.vector.tensor_copy(
                                kt_big[:, ec, j * 256 : j * 256 + 256], ps[:]
                            )
                        else:
                            nc.scalar.copy(
                                kt_big[:, ec, j * 256 : j * 256 + 256], ps[:]
                            )
                    if j + 1 < nstr:
                        kts.append(
                            xt_strip(xk_d, krow0 + (j + 1) * 256, 256, f"xkt{p}_{j+1}")
                        )
                    for h in range(2):
                        for eq in range(4):
                            ps = pspool.tile([128, 256], F32, tag="ps", name="vps")
                            for dc in range(D // 128):
                                nc.tensor.matmul(
                                    ps[:],
                                    xvt[:, dc, h * 128 : h * 128 + 128],
                                    wvq[eq][:, dc, :],
                                    start=(dc == 0),
                                    stop=(dc == D // 128 - 1),
                                )
                            if eq % 2 == 0:
                                nc.vector.tensor_copy(
                                    v_big[:, 2 * j + h, eq * 256 : eq * 256 + 256], ps[:]
                                )
                            else:
                                nc.scalar.copy(
                                    v_big[:, 2 * j + h, eq * 256 : eq * 256 + 256], ps[:]
                                )
                    if j + 1 < nstr:
                        vts.append(
                            xt_strip(xv_d, krow0 + (j + 1) * 256, 256, f"xvt{p}_{j+1}")
                        )
                return kt_big, v_big

            # ---- Q projection + Q^T (strip-pipelined: transposes for strip
            # j+1 are emitted before strip j's matmuls so the DVE copies run
            # under the PE matmuls) -----------------------------------------
            xqts = [xt_strip(xq_d, 0, 256, "xqt0")]
            wqq = wload(wq_d, "wq")
            for j in range(QL // 256):
                if j + 1 < QL // 256:
                    xqts.append(xt_strip(xq_d, (j + 1) * 256, 256, f"xqt{j+1}"))
                xqt = xqts[j]
                for h in range(2):
                    qt = 2 * j + h
                    qrow = outpool.tile([128, E], F32R, tag="out", name=f"q{qt}")
                    for eq in range(4):
                        ps = pspool.tile([128, 256], F32, tag="ps", name="qps")
                        for dc in range(D // 128):
                            nc.tensor.matmul(
                                ps[:],
                                xqt[:, dc, h * 128 : h * 128 + 128],
                                wqq[eq][:, dc, :],
                                start=(dc == 0),
                                stop=(dc == D // 128 - 1),
                            )
                        nc.vector.tensor_copy(qrow[:, eq * 256 : eq * 256 + 256], ps[:])
                    nc.sync.dma_start(qo_d[qt * 128 : qt * 128 + 128, :], qrow[:])
                    for ec in range(E // 128):
                        pst = pspool.tile([128, 256], F32R, tag="ps", name="qtt")
                        nc.tensor.transpose(
                            pst[:, :128], qrow[:, ec * 128 : ec * 128 + 128], ident[:]
                        )
                        nc.vector.tensor_copy(
                            qt_big[:, ec, qt * 128 : qt * 128 + 128], pst[:, :128]
                        )

            wkq = wload(wk_d, "wk")
            wvq = wload(wv_d, "wv")
            if _rep == 0:
                nc.sync.dma_start(maskt[:], mk_d[:].rearrange("m p q -> p m q"))
            for p in range(NPASS):
                kt_big, v_big = kv_proj_pass(p)
                # ---- attention: pairs against this pass's kv chunks ------
                for u in range(4 if phase == "full" else 0):
                    lo, hi = p * PASS_KC, min(4 * u + 4, (p + 1) * PASS_KC)
                    if lo >= hi:
                        continue
                    a_ps = [
                        psapool.tile([128, E], F32, tag="aps", name=f"a{u}_{st}")
                        for st in range(2)
                    ]
                    l_ps = [
                        pslpool.tile([128, 2], F32, tag="lps", name=f"l{u}_{st}")
                        for st in range(2)
                    ]
                    def st_mm(kc):
                        kcl = kc - p * PASS_KC
                        sps = pspool.tile([128, 256], F32, tag="ps", name="sps")
                        for ec in range(E // 128):
                            nc.tensor.matmul(
                                sps[:],
                                kt_big[:, ec, kcl * 128 : kcl * 128 + 128],
                                qt_big[:, ec, u * 256 : u * 256 + 256],
                                start=(ec == 0),
                                stop=(ec == E // 128 - 1),
                            )
                        pt = ptpool.tile([128, 256], F32R, tag="pt", name="pt")
                        nc.scalar.activation(
                            pt[:], sps[:], mybir.ActivationFunctionType.Exp,
                            scale=SCALE,
                        )
                        m = kc - 4 * u
                        if m >= 0:
                            nc.vector.tensor_mul(pt[:], pt[:], maskt[:, m, :])
                        return pt

                    pts = {lo: st_mm(lo)}
                    for kc in range(lo, hi):
                        if kc + 1 < hi:
                            pts[kc + 1] = st_mm(kc + 1)
                        pt = pts.pop(kc)
                        kcl = kc - p * PASS_KC
                        first = kc == lo
                        last = kc == hi - 1
                        for st in range(2):
                            nc.tensor.matmul(
                                l_ps[st][:],
                                pt[:, st * 128 : st * 128 + 128],
                                ones[:],
                                start=first,
                                stop=last,
                            )
                            for eq in range(4):
                                # A tile spans 2 PSUM banks (2 quarters per
                                # bank); start=True zeroes the whole bank, so
                                # only the first quarter of each bank may set
                                # it at the opening chunk.
                                nc.tensor.matmul(
                                    a_ps[st][:, eq * 256 : eq * 256 + 256],
                                    pt[:, st * 128 : st * 128 + 128],
                                    v_big[:, kcl, eq * 256 : eq * 256 + 256],
                                    start=first and eq % 2 == 0,
                                    stop=last,
                                )
                    if u >= 2:
                        # pairs spanning both passes: emit raw partials,
                        # host combines (A0+A1)/(l0+l1)
                        part_d = a0_d if p == 0 else ao_d
                        roff = (2 * (u - 2)) * 128 if p == 0 else (2 * u) * 128
                        lpart_d = l0_d if p == 0 else l1_d
                        for st in range(2):
                            at = outpool.tile([128, E], F32, tag="out", name="at")
                            nc.vector.tensor_copy(at[:], a_ps[st][:])
                            nc.sync.dma_start(
                                part_d[roff + st * 128 : roff + (st + 1) * 128, :],
                                at[:],
                            )
                        ls = smallpool.tile([128, 2], F32, tag="ls", name="ls")
                        nc.vector.tensor_copy(ls[:, 0:1], l_ps[0][:, 0:1])
                        nc.vector.tensor_copy(ls[:, 1:2], l_ps[1][:, 0:1])
                        nc.sync.dma_start(
                            lpart_d[u - 2].rearrange("s p -> p s"), ls[:]
                        )
                    else:
                        # pair complete in pass 0: att rows = A / l
                        for st in range(2):
                            lr = smallpool.tile([128, 1], F32, tag="lr", name="lr")
                            at = outpool.tile([128, E], F32, tag="out", name="at")
                            nc.vector.reciprocal(lr[:], l_ps[st][:, 0:1])
                            nc.vector.tensor_scalar_mul(at[:], a_ps[st][:], lr[:])
                            qt = 2 * u + st
                            nc.sync.dma_start(
                                ao_d[qt * 128 : qt * 128 + 128, :], at[:]
                            )


def _shard_masks(s: int) -> np.ndarray:
    kr = np.arange(128)[:, None]
    qr = np.arange(256)[None, :]
    out = np.empty((4, 128, 256), np.float32)
    for m in range(4):
        out[m] = (m * 128 + kr <= (2 * (qr // 128) + s) * 128 + (qr % 128)).astype(
            np.float32
        )
    return out


def _qidx(s: int) -> np.ndarray:
    ql = np.arange(QL)
    return (2 * (ql // 128) + s) * 128 + (ql % 128)


def make_core_inputs(xq_local, xk, xv, wq, wk, wv, s):
    return {
        "xq": np.ascontiguousarray(xq_local),
        "xk": xk,
        "xv": xv,
        "wq": wq,
        "wk": wk,
        "wv": wv,
        "masks": _shard_masks(s),
    }


_NC_CACHE = {}


def kernel(inputs_for_keys, inputs_for_values, inputs_for_queries, WK, WV, WQ):
    if "nc" not in _NC_CACHE:
        _NC_CACHE["nc"] = build_nc(1)
    nc = _NC_CACHE["nc"]

    xk = np.ascontiguousarray(inputs_for_keys, np.float32)
    xv = np.ascontiguousarray(inputs_for_values, np.float32)
    xq = np.ascontiguousarray(inputs_for_queries, np.float32)
    wk = np.ascontiguousarray(WK, np.float32)
    wv = np.ascontiguousarray(WV, np.float32)
    wq = np.ascontiguousarray(WQ, np.float32)

    idx = [_qidx(0), _qidx(1)]
    msk = [_shard_masks(0), _shard_masks(1)]
    in_maps = []
    for c in range(NCORES):
        b, s = c // 2, c % 2
        in_maps.append(
            {
                "xq": np.ascontiguousarray(xq[b][idx[s]]),
                "xk": xk[b],
                "xv": xv[b],
                "wq": wq,
                "wk": wk,
                "wv": wv,
                "masks": msk[s],
            }
        )
    res = run_bass_kernel_spmd(nc, in_maps, list(range(NCORES)))
    q_full = np.empty((B, S, E), np.float32)
    a_full = np.empty((B, S, E), np.float32)
    for c in range(NCORES):
        b, s = c // 2, c % 2
        r = res.results[c]
        att = r["att_out"].copy()
        # rows 512: of att_out hold pass-1 partials of pairs 2,3
        l0, l1 = r["l0_out"], r["l1_out"]  # [2 pairs, 2 st, 128 q]
        for u in (2, 3):
            for st in range(2):
                rows = slice((2 * u + st) * 128, (2 * u + st + 1) * 128)
                rows0 = slice((2 * (u - 2) + st) * 128, (2 * (u - 2) + st + 1) * 128)
                lsum = l0[u - 2, st] + l1[u - 2, st]
                att[rows] = (r["a0_out"][rows0] + att[rows]) / lsum[:, None]
        q_full[b][idx[s]] = r["q_out"]
        a_full[b][idx[s]] = att
    return q_full, a_full



# revision 7
# speedup vs baseline: 1.7607x; 1.7607x over previous
"""Single-head causal attention on 8 Trainium2 NeuronCores (Bass/Tile).

Problem: B=4, S=2048, D=E=1024 fp32.
  K = Xk @ WK; V = Xv @ WV; Q = Xq @ WQ
  att = softmax(causal(Q K^T / sqrt(S))) @ V;  returns (Q, att)

Sharding (uniform SPMD, per-core differences are data only):
  core c -> batch b = c // 2, kv parity s = c % 2.
  Each core handles ALL 2048 queries of its batch but only its parity
  half of the 16 kv chunks (abs chunk 2j+s for local j in [0,8)).  It
  emits flash-style partials (A = P~V sums, l = P~ row sums) and the
  host combines: att = (A0+A1)/(l0+l1).  This halves the K/V projection
  per core (the baseline duplicated it) at the cost of duplicating the
  cheaper Q projection.

Per-core kernel (all matmul inputs bf16, psum f32, ap<=512):
  - Q phase: per 128-row q-tile: transpose Xq tile (PE), project
    Q = Xq W q (ap512), emit Q rows (bf16), re-transpose to Q^T.
  - KV phase: per local chunk: transpose Xk/Xv rows, V = Xv Wv (ap512);
    per 512-col strip: K^T = Wk^T Xk^T (ap512).
  - Attention: per 256-col q-tile t, local chunks j<=t:
    S^T[k,q] = K^T chunk . Q^T (ap256), P~ = exp(scale*S^T) (bf16),
    causal mask (multiplicative, only at j==t, shift-invariant),
    l += P~^T 1 (ap2), A += P~^T V chunk (ap512).
  - Emission is software-pipelined so PE transposes/matmuls cover the
    DVE/scalar psum-drain latencies (PE p-state drops on any idle gap).
"""

import math
import sys

sys.path.insert(0, "/opt/trn_rl_repo")

import numpy as np  # noqa: E402
import ml_dtypes  # noqa: E402

import concourse.bass as bass  # noqa: E402
import concourse.tile as tile  # noqa: E402
from concourse import bacc, mybir  # noqa: E402
from concourse.bass_utils import run_bass_kernel_spmd  # noqa: E402
from concourse.masks import make_identity  # noqa: E402

B, S, D, E = 4, 2048, 1024, 1024
NCORES = 8
SCALE = 1.0 / math.sqrt(float(S))
F32 = mybir.dt.float32
BF16 = mybir.dt.bfloat16
NPB = np.dtype(ml_dtypes.bfloat16)

NQT = S // 128  # 16 q row-tiles
NKC = 8  # local kv chunks (parity half of 16)
NAT = S // 256  # 8 attention q-tiles (256 q cols each)


def build_nc(reps: int = 1):
    nc = bacc.Bacc("TRN2", target_bir_lowering=False, debug=False, num_devices=NCORES)

    xq_d = nc.dram_tensor("xq", [S, D], BF16, kind="ExternalInput").ap()
    xk_d = nc.dram_tensor("xk", [NKC * 128, D], BF16, kind="ExternalInput").ap()
    xv_d = nc.dram_tensor("xv", [NKC * 128, D], BF16, kind="ExternalInput").ap()
    wq_d = nc.dram_tensor("wq", [D, E], BF16, kind="ExternalInput").ap()
    wk_d = nc.dram_tensor("wk", [D, E], BF16, kind="ExternalInput").ap()
    wv_d = nc.dram_tensor("wv", [D, E], BF16, kind="ExternalInput").ap()
    mk_d = nc.dram_tensor("mask", [128, 256], BF16, kind="ExternalInput").ap()
    qo_d = nc.dram_tensor("q_out", [S, E], BF16, kind="ExternalOutput").ap()
    ao_d = nc.dram_tensor("a_out", [S, E], F32, kind="ExternalOutput").ap()
    lo_d = nc.dram_tensor("l_out", [NAT, 2, 128], F32, kind="ExternalOutput").ap()

    with tile.TileContext(nc) as tc:
        _emit(tc, reps, xq_d, xk_d, xv_d, wq_d, wk_d, wv_d, mk_d, qo_d, ao_d, lo_d)
    nc.compile()
    return nc


def _emit(tc, reps, xq_d, xk_d, xv_d, wq_d, wk_d, wv_d, mk_d, qo_d, ao_d, lo_d):
    nc = tc.nc
    with (
        tc.tile_pool(name="const", bufs=1) as cpool,
        tc.tile_pool(name="wp", bufs=1) as wpool,
        tc.tile_pool(name="big", bufs=1) as bigpool,
        tc.tile_pool(name="xload", bufs=4) as xlpool,
        tc.tile_pool(name="xt", bufs=3) as xtpool,
        tc.tile_pool(name="qrow", bufs=2) as qrowpool,
        tc.tile_pool(name="pt", bufs=3) as ptpool,
        tc.tile_pool(name="outp", bufs=2) as outpool,
        tc.tile_pool(name="ls", bufs=2) as lspool,
    ):
        # ---- constants -------------------------------------------------
        ident_f = cpool.tile([128, 128], F32)
        make_identity(nc, ident_f[:])
        ident = cpool.tile([128, 128], BF16)
        nc.vector.tensor_copy(ident[:], ident_f[:])
        ones_f = cpool.tile([128, 2], F32)
        nc.vector.memset(ones_f[:], 1.0)
        ones = cpool.tile([128, 2], BF16)
        nc.vector.tensor_copy(ones[:], ones_f[:])
        maskt = cpool.tile([128, 256], BF16)

        # ---- weights (per-dc-chunk DMAs so the first matmul starts early)
        def wload(w_d, nm):
            t = wpool.tile([128, D // 128, E], BF16, tag=f"w{nm}", name=f"w{nm}")
            wr = w_d.rearrange("(c p) e -> p c e", p=128)
            for dc in range(D // 128):
                nc.sync.dma_start(t[:, dc], wr[:, dc])
            return t

# big persistent tensors
        qt_big = bigpool.tile([128, E // 128, S], BF16, tag="qt", name="qt_big")
        kt_big = bigpool.tile([128, E // 128, NKC * 128], BF16, tag="kt", name="kt")
        v_big = bigpool.tile([128, NKC, E], BF16, tag="v", name="v")
        xkt_big = bigpool.tile(
            [128, D // 128, NKC * 128], BF16, tag="xkt", name="xkt"
        )

        for _rep in range(reps):
            # PSUM pools: proj phases use trpool+prpool (4 banks), the
            # attention block below uses its own 8 banks after these close.
            with (
                tc.tile_pool(name="trp", bufs=2, space="PSUM") as trpool,
                tc.tile_pool(name="prp", bufs=2, space="PSUM") as prpool,
            ):
                # ============ Q phase: project + transpose ==============
                def xq_tr(i):
                    """Load xq tile i, return its transpose [128, 8dc, 128]."""
                    xl = xlpool.tile([128, D], BF16, tag="xl", name=f"xql{i}")
                    nc.sync.dma_start(xl[:], xq_d[i * 128 : (i + 1) * 128, :])
                    xt = xtpool.tile(
                        [128, D // 128, 128], BF16, tag="xqt", name=f"xqt{i}"
                    )
                    for dc in range(D // 128):
                        pst = trpool.tile([128, 128], BF16, tag="tr", name="trq")
                        nc.tensor.transpose(
                            pst[:], xl[:, dc * 128 : (dc + 1) * 128], ident[:]
                        )
                        nc.vector.tensor_copy(xt[:, dc], pst[:])
                    return xt

                # first two x-tile DMAs go out before the weight DMAs so
                # the PE's first transposes start ~1us in
                xqts = {0: xq_tr(0), 1: xq_tr(1)}
                wq_sb = wload(wq_d, "q")
                for i in range(NQT):
                    xt = xqts.pop(i)
                    qrow = qrowpool.tile([128, E], BF16, tag="qrow", name=f"q{i}")
                    for eh in range(2):
                        pr = prpool.tile([128, 512], F32, tag="pr", name="prq")
                        for dc in range(D // 128):
                            nc.tensor.matmul(
                                pr[:],
                                xt[:, dc],
                                wq_sb[:, dc, eh * 512 : (eh + 1) * 512],
                                start=(dc == 0),
                                stop=(dc == D // 128 - 1),
                            )
                        if eh == 0:
                            nc.vector.tensor_copy(qrow[:, :512], pr[:])
                        else:
                            nc.scalar.copy(qrow[:, 512:], pr[:])
                    nc.sync.dma_start(qo_d[i * 128 : (i + 1) * 128, :], qrow[:])
                    # prefetch xq transpose i+2 between Q matmuls and Q^T
                    # transposes: covers the psum->qrow drain latency on PE
                    # and keeps the xl DMA two iterations ahead of its use
                    if i + 2 < NQT:
                        xqts[i + 2] = xq_tr(i + 2)
                    for ec in range(E // 128):
                        pst = trpool.tile([128, 128], BF16, tag="tr", name="trq2")
                        nc.tensor.transpose(
                            pst[:], qrow[:, ec * 128 : (ec + 1) * 128], ident[:]
                        )
                        nc.vector.tensor_copy(
                            qt_big[:, ec, i * 128 : (i + 1) * 128], pst[:]
                        )

                # ============ KV phase ==================================
                wk_sb = wload(wk_d, "k")
                wv_sb = wload(wv_d, "v")
                if _rep == 0:
                    nc.sync.dma_start(maskt[:], mk_d[:])

                def kv_tr(j):
                    """Transpose xk chunk j into xkt_big; return xv^T chunk."""
                    xkl = xlpool.tile([128, D], BF16, tag="xl", name=f"xkl{j}")
                    nc.sync.dma_start(xkl[:], xk_d[j * 128 : (j + 1) * 128, :])
                    xvl = xlpool.tile([128, D], BF16, tag="xl", name=f"xvl{j}")
                    nc.sync.dma_start(xvl[:], xv_d[j * 128 : (j + 1) * 128, :])
                    for dc in range(D // 128):
                        pst = trpool.tile([128, 128], BF16, tag="tr", name="trk")
                        nc.tensor.transpose(
                            pst[:], xkl[:, dc * 128 : (dc + 1) * 128], ident[:]
                        )
                        nc.vector.tensor_copy(
                            xkt_big[:, dc, j * 128 : (j + 1) * 128], pst[:]
                        )
                    xvt = xtpool.tile(
                        [128, D // 128, 128], BF16, tag="xvt", name=f"xvt{j}"
                    )
                    for dc in range(D // 128):
                        pst = trpool.tile([128, 128], BF16, tag="tr", name="trv")
                        nc.tensor.transpose(
                            pst[:], xvl[:, dc * 128 : (dc + 1) * 128], ident[:]
                        )
                        nc.vector.tensor_copy(xvt[:, dc], pst[:])
                    return xvt

                def v_proj(j, xvt):
                    for eh in range(2):
                        pr = prpool.tile([128, 512], F32, tag="pr", name="prv")
                        for dc in range(D // 128):
                            nc.tensor.matmul(
                                pr[:],
                                xvt[:, dc],
                                wv_sb[:, dc, eh * 512 : (eh + 1) * 512],
                                start=(dc == 0),
                                stop=(dc == D // 128 - 1),
                            )
                        if eh == 0:
                            nc.vector.tensor_copy(v_big[:, j, :512], pr[:])
                        else:
                            nc.scalar.copy(v_big[:, j, 512:], pr[:])

                xvts = {}
                for h in range(2):
                    for j in range(4 * h, 4 * h + 4):
                        xvts[j] = kv_tr(j)
                        if j - 1 in xvts:
                            v_proj(j - 1, xvts.pop(j - 1))
                    for ec in range(E // 128):
                        pr = prpool.tile([128, 512], F32, tag="pr", name="prk")
                        for dc in range(D // 128):
                            nc.tensor.matmul(
                                pr[:],
                                wk_sb[:, dc, ec * 128 : (ec + 1) * 128],
                                xkt_big[:, dc, h * 512 : (h + 1) * 512],
                                start=(dc == 0),
                                stop=(dc == D // 128 - 1),
                            )
                        if ec % 2 == 0:
                            nc.vector.tensor_copy(
                                kt_big[:, ec, h * 512 : (h + 1) * 512], pr[:]
                            )
                        else:
                            nc.scalar.copy(
                                kt_big[:, ec, h * 512 : (h + 1) * 512], pr[:]
                            )
                v_proj(7, xvts.pop(7))

            # ============ attention =====================================
            with (
                tc.tile_pool(name="spsp", bufs=2, space="PSUM") as spspool,
                tc.tile_pool(name="psap", bufs=4, space="PSUM") as psapool,
                tc.tile_pool(name="pslp", bufs=2, space="PSUM") as pslpool,
            ):
                for t in range(NAT):
                    a_ps = [
                        psapool.tile([128, 512], F32, tag="aps", name=f"a{t}_{q}")
                        for q in range(4)
                    ]
                    l_ps = [
                        pslpool.tile([128, 2], F32, tag="lps", name=f"l{t}_{st}")
                        for st in range(2)
                    ]

                    def st_mm(j, t=t):
                        sps = spspool.tile([128, 256], F32, tag="sps", name="sps")
                        for ec in range(E // 128):
                            nc.tensor.matmul(
                                sps[:],
                                kt_big[:, ec, j * 128 : (j + 1) * 128],
                                qt_big[:, ec, t * 256 : (t + 1) * 256],
                                start=(ec == 0),
                                stop=(ec == E // 128 - 1),
                            )
                        pt = ptpool.tile([128, 256], BF16, tag="pt", name="pt")
                        nc.scalar.activation(
                            pt[:], sps[:], mybir.ActivationFunctionType.Exp,
                            scale=SCALE,
                        )
                        if j == t:
                            nc.vector.tensor_mul(pt[:], pt[:], maskt[:])
                        return pt

                    pts = {0: st_mm(0)}
                    for j in range(t + 1):
                        if j + 1 <= t:
                            pts[j + 1] = st_mm(j + 1)
                        pt = pts.pop(j)
                        first, last = j == 0, j == t
                        for st in range(2):
                            nc.tensor.matmul(
                                l_ps[st][:],
                                pt[:, st * 128 : (st + 1) * 128],
                                ones[:],
                                start=first,
                                stop=last,
                            )
                            for eh in range(2):
                                nc.tensor.matmul(
                                    a_ps[2 * st + eh][:],
                                    pt[:, st * 128 : (st + 1) * 128],
                                    v_big[:, j, eh * 512 : (eh + 1) * 512],
                                    start=first,
                                    stop=last,
                                )
                    ls = lspool.tile([128, 2], F32, tag="ls", name="ls")
                    for st in range(2):
                        at = outpool.tile([128, E], F32, tag="at", name="at")
                        nc.vector.tensor_copy(at[:, :512], a_ps[2 * st][:])
                        nc.scalar.copy(at[:, 512:], a_ps[2 * st + 1][:])
                        nc.sync.dma_start(
                            ao_d[t * 256 + st * 128 : t * 256 + (st + 1) * 128, :],
                            at[:],
                        )
                        nc.vector.tensor_copy(ls[:, st : st + 1], l_ps[st][:, 0:1])
                    nc.sync.dma_start(lo_d[t].rearrange("s p -> p s"), ls[:])


def _mask(s: int) -> np.ndarray:
    kr = np.arange(128)[:, None]
    qr = np.arange(256)[None, :]
    return (s * 128 + kr <= qr).astype(NPB)


def make_core_inputs(xq_b, xk_b, xv_b, wq, wk, wv, s):
    """Per-core input dict. xq_b/xk_b/xv_b are the batch's [S, D] arrays."""
    return {
        "xq": np.ascontiguousarray(xq_b.astype(NPB)),
        "xk": np.ascontiguousarray(
            xk_b.reshape(16, 128, D)[s::2].reshape(NKC * 128, D).astype(NPB)
        ),
        "xv": np.ascontiguousarray(
            xv_b.reshape(16, 128, D)[s::2].reshape(NKC * 128, D).astype(NPB)
        ),
        "wq": wq,
        "wk": wk,
        "wv": wv,
        "mask": _mask(s),
    }


_NC_CACHE = {}


def kernel(inputs_for_keys, inputs_for_values, inputs_for_queries, WK, WV, WQ):
    if "nc" not in _NC_CACHE:
        _NC_CACHE["nc"] = build_nc(1)
    nc = _NC_CACHE["nc"]

    xk = np.asarray(inputs_for_keys, np.float32)
    xv = np.asarray(inputs_for_values, np.float32)
    xq = np.asarray(inputs_for_queries, np.float32)
    wk = np.ascontiguousarray(np.asarray(WK, np.float32).astype(NPB))
    wv = np.ascontiguousarray(np.asarray(WV, np.float32).astype(NPB))
    wq = np.ascontiguousarray(np.asarray(WQ, np.float32).astype(NPB))

    in_maps = []
    for c in range(NCORES):
        b, s = c // 2, c % 2
        in_maps.append(make_core_inputs(xq[b], xk[b], xv[b], wq, wk, wv, s))
    res = run_bass_kernel_spmd(nc, in_maps, list(range(NCORES)))

    q_full = np.empty((B, S, E), np.float32)
    a_full = np.empty((B, S, E), np.float32)
    for b in range(B):
        r0 = res.results[2 * b]
        r1 = res.results[2 * b + 1]
        a = r0["a_out"] + r1["a_out"]
        l = (r0["l_out"] + r1["l_out"]).reshape(S)
        a_full[b] = a / l[:, None]
        q_full[b] = r0["q_out"].astype(np.float32)
    return q_full, a_full


# revision 8
# speedup vs baseline: 1.9063x; 1.0827x over previous
"""Single-head causal attention on 8 Trainium2 NeuronCores (Bass/Tile).

Problem: B=4, S=2048, D=E=1024 fp32.
  K = Xk @ WK; V = Xv @ WV; Q = Xq @ WQ
  att = softmax(causal(Q K^T / sqrt(S))) @ V;  returns (Q, att)

Sharding (uniform SPMD, per-core differences are data only):
  core c -> batch b = c // 2, kv parity s = c % 2.
  Each core handles ALL 2048 queries of its batch but only its parity
  half of the 16 kv chunks (abs chunk 2j+s for local j in [0,8)).  It
  emits flash-style partials (A = P~V sums, l = P~ row sums) and the
  host combines: att = (A0+A1)/(l0+l1).  This halves the K/V projection
  per core (a batch-pair split would duplicate it) at the cost of
  duplicating the cheaper Q projection.

Per-core kernel (matmul inputs bf16, psum f32, ap<=512):
  - Q phase: per 128-row q-tile: transpose Xq tile (PE), project
    Q = Xq Wq (ap512), emit Q rows (bf16), re-transpose to Q^T.
  - KV phase: per local chunk: transpose Xk/Xv rows, V = Xv Wv (ap512);
    per 512-col strip: K^T = Wk^T Xk^T (ap512).
  - Attention: per 256-col q-tile t, local chunks j<=t:
    S^T[k,q] = K^T chunk . Q^T (ap256), P~ = exp(scale*S^T) (bf16),
    causal mask (multiplicative, only at j==t, shift-invariant),
    l += P~^T 1 (ap2), A += P~^T V chunk (ap512).
  - Emission is software-pipelined so PE transposes/matmuls cover the
    DVE/scalar psum-drain latencies (PE p-state drops on any idle gap).
  - DMAs are batched into ~1MB transfers (the per-descriptor queue cost
    is ~1us; 175 small DMAs saturated the Sync queue).  Loads go on the
    Sync queue, stores on the otherwise-idle GpSimd queue.  Constants
    (identity/ones/mask) arrive via one DMA instead of gpsimd iota.
"""

import math
import sys

sys.path.insert(0, "/opt/trn_rl_repo")

import numpy as np  # noqa: E402
import ml_dtypes  # noqa: E402

import concourse.bass as bass  # noqa: E402
import concourse.tile as tile  # noqa: E402
from concourse import bacc, mybir  # noqa: E402
from concourse.bass_utils import run_bass_kernel_spmd  # noqa: E402

B, S, D, E = 4, 2048, 1024, 1024
NCORES = 8
SCALE = 1.0 / math.sqrt(float(S))
F32 = mybir.dt.float32
BF16 = mybir.dt.bfloat16
NPB = np.dtype(ml_dtypes.bfloat16)

NQT = S // 128  # 16 q row-tiles
NKC = 8  # local kv chunks (parity half of 16)
NAT = S // 256  # 8 attention q-tiles (256 q cols each)
NCON = 128 + 2 + 256  # ident | ones | mask columns


def build_nc(reps: int = 1):
    nc = bacc.Bacc("TRN2", target_bir_lowering=False, debug=False, num_devices=NCORES)

    xq_d = nc.dram_tensor("xq", [S, D], BF16, kind="ExternalInput").ap()
    xk_d = nc.dram_tensor("xk", [NKC * 128, D], BF16, kind="ExternalInput").ap()
    xv_d = nc.dram_tensor("xv", [NKC * 128, D], BF16, kind="ExternalInput").ap()
    wq_d = nc.dram_tensor("wq", [D, E], BF16, kind="ExternalInput").ap()
    wk_d = nc.dram_tensor("wk", [D, E], BF16, kind="ExternalInput").ap()
    wv_d = nc.dram_tensor("wv", [D, E], BF16, kind="ExternalInput").ap()
    cn_d = nc.dram_tensor("consts", [128, NCON], BF16, kind="ExternalInput").ap()
    qo_d = nc.dram_tensor("q_out", [S, E], BF16, kind="ExternalOutput").ap()
    ao_d = nc.dram_tensor("a_out", [S, E], F32, kind="ExternalOutput").ap()
    lo_d = nc.dram_tensor("l_out", [NAT, 2, 128], F32, kind="ExternalOutput").ap()

    with tile.TileContext(nc) as tc:
        _emit(tc, reps, xq_d, xk_d, xv_d, wq_d, wk_d, wv_d, cn_d, qo_d, ao_d, lo_d)
    nc.compile()
    return nc


def _emit(tc, reps, xq_d, xk_d, xv_d, wq_d, wk_d, wv_d, cn_d, qo_d, ao_d, lo_d):
    nc = tc.nc
    with (
        tc.tile_pool(name="const", bufs=1) as cpool,
        tc.tile_pool(name="wp", bufs=2) as wpool,
        tc.tile_pool(name="big", bufs=1) as bigpool,
        tc.tile_pool(name="xq2", bufs=2) as xq2pool,
        tc.tile_pool(name="kv2", bufs=2) as kv2pool,
        tc.tile_pool(name="xt", bufs=3) as xtpool,
        tc.tile_pool(name="qrow", bufs=2) as qrowpool,
        tc.tile_pool(name="pt", bufs=3) as ptpool,
        tc.tile_pool(name="outp", bufs=2) as outpool,
    ):
        # ---- constants: one DMA (ident | ones | mask) ------------------
        consts = cpool.tile([128, NCON], BF16)
        nc.sync.dma_start(consts[:], cn_d[:])
        ident = consts[:, 0:128]
        ones = consts[:, 128:130]
        maskt = consts[:, 130:386]
        ls_big = cpool.tile([128, NAT, 2], F32)

        # weights: 2 DMAs each; wq/wk/wv rotate through 2 slots (wv
        # reuses wq's slot after the Q phase finishes with it)
        def wload(w_d, nm):
            t = wpool.tile([128, D // 128, E], BF16, tag="w", name=f"w{nm}")
            wr = w_d.rearrange("(c p) e -> p c e", p=128)
            for dh in range(2):
                nc.sync.dma_start(t[:, 4 * dh : 4 * dh + 4], wr[:, 4 * dh : 4 * dh + 4])
            return t

        # big persistent tensors
        qt_big = bigpool.tile([128, E // 128, S], BF16, tag="qt", name="qt_big")
        kt_big = bigpool.tile([128, E // 128, NKC * 128], BF16, tag="kt", name="kt")
        v_big = bigpool.tile([128, NKC, E], BF16, tag="v", name="v")
        xkt_big = bigpool.tile(
            [128, D // 128, NKC * 128], BF16, tag="xkt", name="xkt"
        )

        for _rep in range(reps):
            # PSUM pools: proj phases use trpool+prpool (4 banks); the
            # attention block uses its own 8 banks after these close.
            with (
                tc.tile_pool(name="trp", bufs=2, space="PSUM") as trpool,
                tc.tile_pool(name="prp", bufs=2, space="PSUM") as prpool,
            ):
                # ============ Q phase: project + transpose ==============
                def xq_load(u):
                    xl2 = xq2pool.tile([128, 2, D], BF16, tag="xl2", name=f"xq{u}")
                    nc.sync.dma_start(
                        xl2[:],
                        xq_d[u * 256 : (u + 1) * 256, :].rearrange(
                            "(c p) d -> p c d", p=128
                        ),
                    )
                    return xl2

                def xq_tr(xl2, i):
                    xt = xtpool.tile(
                        [128, D // 128, 128], BF16, tag="xqt", name=f"xqt{i}"
                    )
                    for dc in range(D // 128):
                        pst = trpool.tile([128, 128], BF16, tag="tr", name="trq")
                        nc.tensor.transpose(
                            pst[:], xl2[:, i % 2, dc * 128 : (dc + 1) * 128], ident
                        )
                        nc.vector.tensor_copy(xt[:, dc], pst[:])
                    return xt

                xls = {0: xq_load(0)}
                xts = {0: xq_tr(xls[0], 0), 1: xq_tr(xls[0], 1)}
                wq_sb = wload(wq_d, "q")
                qrow2 = None
                for i in range(NQT):
                    u, c = divmod(i, 2)
                    xt = xts.pop(i)
                    if c == 0:
                        qrow2 = qrowpool.tile(
                            [128, 2, E], BF16, tag="qrow", name=f"q{u}"
                        )
                    for eh in range(2):
                        pr = prpool.tile([128, 512], F32, tag="pr", name="prq")
                        for dc in range(D // 128):
                            nc.tensor.matmul(
                                pr[:],
                                xt[:, dc],
                                wq_sb[:, dc, eh * 512 : (eh + 1) * 512],
                                start=(dc == 0),
                                stop=(dc == D // 128 - 1),
                            )
                        if eh == 0:
                            nc.vector.tensor_copy(
                                qrow2[:, c, :512], pr[:]
                            )
                        else:
                            nc.scalar.copy(qrow2[:, c, 512:], pr[:])
                    if c == 1:
                        nc.gpsimd.dma_start(
                            qo_d[u * 256 : (u + 1) * 256, :].rearrange(
                                "(c p) e -> p c e", p=128
                            ),
                            qrow2[:],
                        )
                    # prefetch xq transpose i+2 between Q matmuls and Q^T
                    # transposes: covers the psum->qrow drain latency on PE
                    # and keeps the xl DMA a pair ahead of its use
                    if i + 2 < NQT:
                        if (i + 2) % 2 == 0:
                            xls[u + 1] = xq_load(u + 1)
                            xls.pop(u - 1, None)
                        xts[i + 2] = xq_tr(xls[(i + 2) // 2], i + 2)
                    for ec in range(E // 128):
                        pst = trpool.tile([128, 128], BF16, tag="tr", name="trq2")
                        nc.tensor.transpose(
                            pst[:], qrow2[:, c, ec * 128 : (ec + 1) * 128], ident
                        )
                        nc.vector.tensor_copy(
                            qt_big[:, ec, i * 128 : (i + 1) * 128], pst[:]
                        )

                # ============ KV phase ==================================
                wk_sb = wload(wk_d, "k")
                wv_sb = wload(wv_d, "v")

                def kv_load(g):
                    """Load xk/xv chunk pair g (chunks 2g, 2g+1)."""
                    xk2 = kv2pool.tile([128, 2, D], BF16, tag="xk2", name=f"xk{g}")
                    nc.sync.dma_start(
                        xk2[:],
                        xk_d[g * 256 : (g + 1) * 256, :].rearrange(
                            "(c p) d -> p c d", p=128
                        ),
                    )
                    xv2 = kv2pool.tile([128, 2, D], BF16, tag="xv2", name=f"xv{g}")
                    nc.sync.dma_start(
                        xv2[:],
                        xv_d[g * 256 : (g + 1) * 256, :].rearrange(
                            "(c p) d -> p c d", p=128
                        ),
                    )
                    return xk2, xv2

                def kv_tr(kvl, j):
                    """Transpose xk chunk j into xkt_big; return xv^T chunk."""
                    xk2, xv2 = kvl
                    for dc in range(D // 128):
                        pst = trpool.tile([128, 128], BF16, tag="tr", name="trk")
                        nc.tensor.transpose(
                            pst[:], xk2[:, j % 2, dc * 128 : (dc + 1) * 128], ident
                        )
                        nc.vector.tensor_copy(
                            xkt_big[:, dc, j * 128 : (j + 1) * 128], pst[:]
                        )
                    xvt = xtpool.tile(
                        [128, D // 128, 128], BF16, tag="xvt", name=f"xvt{j}"
                    )
                    for dc in range(D // 128):
                        pst = trpool.tile([128, 128], BF16, tag="tr", name="trv")
                        nc.tensor.transpose(
                            pst[:], xv2[:, j % 2, dc * 128 : (dc + 1) * 128], ident
                        )
                        nc.vector.tensor_copy(xvt[:, dc], pst[:])
                    return xvt

                def v_proj(j, xvt):
                    for eh in range(2):
                        pr = prpool.tile([128, 512], F32, tag="pr", name="prv")
                        for dc in range(D // 128):
                            nc.tensor.matmul(
                                pr[:],
                                xvt[:, dc],
                                wv_sb[:, dc, eh * 512 : (eh + 1) * 512],
                                start=(dc == 0),
                                stop=(dc == D // 128 - 1),
                            )
                        if eh == 0:
                            nc.vector.tensor_copy(v_big[:, j, :512], pr[:])
                        else:
                            nc.scalar.copy(v_big[:, j, 512:], pr[:])

                kvls = {0: kv_load(0)}
                xvts = {}
                for h in range(2):
                    for j in range(4 * h, 4 * h + 4):
                        if j % 2 == 0 and j // 2 + 1 < 4:
                            kvls[j // 2 + 1] = kv_load(j // 2 + 1)
                            kvls.pop(j // 2 - 1, None)
                        xvts[j] = kv_tr(kvls[j // 2], j)
                        if j - 1 in xvts:
                            v_proj(j - 1, xvts.pop(j - 1))
                    for ec in range(E // 128):
                        pr = prpool.tile([128, 512], F32, tag="pr", name="prk")
                        for dc in range(D // 128):
                            nc.tensor.matmul(
                                pr[:],
                                wk_sb[:, dc, ec * 128 : (ec + 1) * 128],
                                xkt_big[:, dc, h * 512 : (h + 1) * 512],
                                start=(dc == 0),
                                stop=(dc == D // 128 - 1),
                            )
                        if ec % 2 == 0:
                            nc.vector.tensor_copy(
                                kt_big[:, ec, h * 512 : (h + 1) * 512], pr[:]
                            )
                        else:
                            nc.scalar.copy(
                                kt_big[:, ec, h * 512 : (h + 1) * 512], pr[:]
                            )
                v_proj(7, xvts.pop(7))

            # ============ attention =====================================
            with (
                tc.tile_pool(name="spsp", bufs=2, space="PSUM") as spspool,
                tc.tile_pool(name="psap", bufs=4, space="PSUM") as psapool,
                tc.tile_pool(name="pslp", bufs=2, space="PSUM") as pslpool,
            ):
                for t in range(NAT):
                    a_ps = [
                        psapool.tile([128, 512], F32, tag="aps", name=f"a{t}_{q}")
                        for q in range(4)
                    ]
                    l_ps = [
                        pslpool.tile([128, 2], F32, tag="lps", name=f"l{t}_{st}")
                        for st in range(2)
                    ]

                    def st_mm(j, t=t):
                        sps = spspool.tile([128, 256], F32, tag="sps", name="sps")
                        for ec in range(E // 128):
                            nc.tensor.matmul(
                                sps[:],
                                kt_big[:, ec, j * 128 : (j + 1) * 128],
                                qt_big[:, ec, t * 256 : (t + 1) * 256],
                                start=(ec == 0),
                                stop=(ec == E // 128 - 1),
                            )
                        pt = ptpool.tile([128, 256], BF16, tag="pt", name="pt")
                        nc.scalar.activation(
                            pt[:], sps[:], mybir.ActivationFunctionType.Exp,
                            scale=SCALE,
                        )
                        if j == t:
                            nc.vector.tensor_mul(pt[:], pt[:], maskt)
                        return pt

                    pts = {0: st_mm(0)}
                    for j in range(t + 1):
                        if j + 1 <= t:
                            pts[j + 1] = st_mm(j + 1)
                        pt = pts.pop(j)
                        first, last = j == 0, j == t
                        for st in range(2):
                            nc.tensor.matmul(
                                l_ps[st][:],
                                pt[:, st * 128 : (st + 1) * 128],
                                ones,
                                start=first,
                                stop=last,
                            )
                            for eh in range(2):
                                nc.tensor.matmul(
                                    a_ps[2 * st + eh][:],
                                    pt[:, st * 128 : (st + 1) * 128],
                                    v_big[:, j, eh * 512 : (eh + 1) * 512],
                                    start=first,
                                    stop=last,
                                )
                    at2 = outpool.tile([128, 2, E], F32, tag="at", name=f"at{t}")
                    for st in range(2):
                        if st == 0:
                            nc.vector.tensor_copy(at2[:, st, :512], a_ps[2 * st][:])
                            nc.scalar.copy(at2[:, st, 512:], a_ps[2 * st + 1][:])
                        else:
                            nc.scalar.copy(at2[:, st, :512], a_ps[2 * st][:])
                            nc.vector.tensor_copy(
                                at2[:, st, 512:], a_ps[2 * st + 1][:]
                            )
                        nc.vector.tensor_copy(
                            ls_big[:, t, st : st + 1], l_ps[st][:, 0:1]
                        )
                    nc.gpsimd.dma_start(
                        ao_d[t * 256 : (t + 1) * 256, :].rearrange(
                            "(s p) e -> p s e", p=128
                        ),
                        at2[:],
                    )
                nc.gpsimd.dma_start(lo_d.rearrange("t s p -> p t s"), ls_big[:])


def _mask(s: int) -> np.ndarray:
    kr = np.arange(128)[:, None]
    qr = np.arange(256)[None, :]
    return (s * 128 + kr <= qr).astype(NPB)


def _consts(s: int) -> np.ndarray:
    out = np.zeros((128, NCON), NPB)
    out[:, :128] = np.eye(128, dtype=np.float32).astype(NPB)
    out[:, 128:130] = 1.0
    out[:, 130:386] = _mask(s)
    return out


def make_core_inputs(xq_b, xk_b, xv_b, wq, wk, wv, s):
    """Per-core input dict. xq_b/xk_b/xv_b are the batch's [S, D] arrays."""
    return {
        "xq": np.ascontiguousarray(xq_b.astype(NPB)),
        "xk": np.ascontiguousarray(
            xk_b.reshape(16, 128, D)[s::2].reshape(NKC * 128, D).astype(NPB)
        ),
        "xv": np.ascontiguousarray(
            xv_b.reshape(16, 128, D)[s::2].reshape(NKC * 128, D).astype(NPB)
        ),
        "wq": wq,
        "wk": wk,
        "wv": wv,
        "consts": _consts(s),
    }


_NC_CACHE = {}


def kernel(inputs_for_keys, inputs_for_values, inputs_for_queries, WK, WV, WQ):
    if "nc" not in _NC_CACHE:
        _NC_CACHE["nc"] = build_nc(1)
    nc = _NC_CACHE["nc"]

    xk = np.asarray(inputs_for_keys, np.float32)
    xv = np.asarray(inputs_for_values, np.float32)
    xq = np.asarray(inputs_for_queries, np.float32)
    wk = np.ascontiguousarray(np.asarray(WK, np.float32).astype(NPB))
    wv = np.ascontiguousarray(np.asarray(WV, np.float32).astype(NPB))
    wq = np.ascontiguousarray(np.asarray(WQ, np.float32).astype(NPB))

    in_maps = []
    for c in range(NCORES):
        b, s = c // 2, c % 2
        in_maps.append(make_core_inputs(xq[b], xk[b], xv[b], wq, wk, wv, s))
    res = run_bass_kernel_spmd(nc, in_maps, list(range(NCORES)))

    q_full = np.empty((B, S, E), np.float32)
    a_full = np.empty((B, S, E), np.float32)
    for b in range(B):
        r0 = res.results[2 * b]
        r1 = res.results[2 * b + 1]
        a = r0["a_out"] + r1["a_out"]
        l = (r0["l_out"] + r1["l_out"]).reshape(S)
        a_full[b] = a / l[:, None]
        q_full[b] = r0["q_out"].astype(np.float32)
    return q_full, a_full


# revision 13
# speedup vs baseline: 1.9065x; 1.0001x over previous
"""Single-head causal attention on 8 Trainium2 NeuronCores (Bass/Tile).

Problem: B=4, S=2048, D=E=1024 fp32.
  K = Xk @ WK; V = Xv @ WV; Q = Xq @ WQ
  att = softmax(causal(Q K^T / sqrt(S))) @ V;  returns (Q, att)

Sharding (uniform SPMD, per-core differences are data only):
  core c -> batch b = c // 2, kv parity s = c % 2.
  Each core handles ALL 2048 queries of its batch but only its parity
  half of the 16 kv chunks (abs chunk 2j+s for local j in [0,8)).  It
  emits flash-style partials (A = P~V sums, l = P~ row sums) and the
  host combines: att = (A0+A1)/(l0+l1).  This halves the K/V projection
  per core (a batch-pair split would duplicate it) at the cost of
  duplicating the cheaper Q projection.

Per-core kernel (matmul inputs bf16, psum f32, ap<=512):
  - Q phase: per 128-row q-tile: transpose Xq tile (PE), project
    Q = Xq Wq (ap512), emit Q rows (bf16), re-transpose to Q^T.
  - KV phase: per local chunk: transpose Xk/Xv rows, V = Xv Wv (ap512);
    per 512-col strip: K^T = Wk^T Xk^T (ap512).
  - Attention: per 256-col q-tile t, local chunks j<=t:
    S^T[k,q] = K^T chunk . Q^T (ap256), P~ = exp(scale*S^T) (bf16),
    causal mask (multiplicative, only at j==t, shift-invariant),
    l += P~^T 1 (ap2), A += P~^T V chunk (ap512).
  - Emission is software-pipelined so PE transposes/matmuls cover the
    DVE/scalar psum-drain latencies (PE p-state drops on any idle gap).
  - DMAs are batched into ~1MB transfers (the per-descriptor queue cost
    is ~1us; 175 small DMAs saturated the Sync queue).  Loads go on the
    Sync queue, stores on the otherwise-idle GpSimd queue.  Constants
    (identity/ones/mask) arrive via one DMA instead of gpsimd iota.
"""

import math
import sys

sys.path.insert(0, "/opt/trn_rl_repo")

import numpy as np  # noqa: E402
import ml_dtypes  # noqa: E402

import concourse.bass as bass  # noqa: E402
import concourse.tile as tile  # noqa: E402
from concourse import bacc, mybir  # noqa: E402
from concourse.bass_utils import run_bass_kernel_spmd  # noqa: E402

B, S, D, E = 4, 2048, 1024, 1024
NCORES = 8
SCALE = 1.0 / math.sqrt(float(S))
F32 = mybir.dt.float32
BF16 = mybir.dt.bfloat16
NPB = np.dtype(ml_dtypes.bfloat16)

NQT = S // 128  # 16 q row-tiles
NKC = 8  # local kv chunks (parity half of 16)
NAT = S // 256  # 8 attention q-tiles (256 q cols each)
NCON = 128 + 2 + 256  # ident | ones | mask columns


def build_nc(reps: int = 1):
    nc = bacc.Bacc("TRN2", target_bir_lowering=False, debug=False, num_devices=NCORES)

    xq_d = nc.dram_tensor("xq", [S, D], BF16, kind="ExternalInput").ap()
    xk_d = nc.dram_tensor("xk", [NKC * 128, D], BF16, kind="ExternalInput").ap()
    xv_d = nc.dram_tensor("xv", [NKC * 128, D], BF16, kind="ExternalInput").ap()
    wq_d = nc.dram_tensor("wq", [D, E], BF16, kind="ExternalInput").ap()
    wk_d = nc.dram_tensor("wk", [D, E], BF16, kind="ExternalInput").ap()
    wv_d = nc.dram_tensor("wv", [D, E], BF16, kind="ExternalInput").ap()
    cn_d = nc.dram_tensor("consts", [128, NCON], BF16, kind="ExternalInput").ap()
    qo_d = nc.dram_tensor("q_out", [S, E], BF16, kind="ExternalOutput").ap()
    ao_d = nc.dram_tensor("a_out", [S, E], BF16, kind="ExternalOutput").ap()
    lo_d = nc.dram_tensor("l_out", [NAT, 2, 128], F32, kind="ExternalOutput").ap()

    with tile.TileContext(nc) as tc:
        _emit(tc, reps, xq_d, xk_d, xv_d, wq_d, wk_d, wv_d, cn_d, qo_d, ao_d, lo_d)
    nc.compile()
    return nc


def _emit(tc, reps, xq_d, xk_d, xv_d, wq_d, wk_d, wv_d, cn_d, qo_d, ao_d, lo_d):
    nc = tc.nc
    with (
        tc.tile_pool(name="const", bufs=1) as cpool,
        tc.tile_pool(name="wp", bufs=2) as wpool,
        tc.tile_pool(name="big", bufs=1) as bigpool,
        tc.tile_pool(name="xq2", bufs=2) as xq2pool,
        tc.tile_pool(name="kv2", bufs=2) as kv2pool,
        tc.tile_pool(name="xt", bufs=3) as xtpool,
        tc.tile_pool(name="qrow", bufs=2) as qrowpool,
        tc.tile_pool(name="pt", bufs=3) as ptpool,
        tc.tile_pool(name="outp", bufs=2) as outpool,
    ):
        # ---- constants: one DMA (ident | ones | mask) ------------------
        consts = cpool.tile([128, NCON], BF16)
        nc.sync.dma_start(consts[:], cn_d[:])
        ident = consts[:, 0:128]
        ones = consts[:, 128:130]
        maskt = consts[:, 130:386]
        ls_big = cpool.tile([128, NAT, 2], F32)

        # weights: 2 DMAs each; wq/wk/wv rotate through 2 slots (wv
        # reuses wq's slot after the Q phase finishes with it)
        def wload(w_d, nm):
            t = wpool.tile([128, D // 128, E], BF16, tag="w", name=f"w{nm}")
            wr = w_d.rearrange("(c p) e -> p c e", p=128)
            for dh in range(2):
                nc.sync.dma_start(t[:, 4 * dh : 4 * dh + 4], wr[:, 4 * dh : 4 * dh + 4])
            return t

        # big persistent tensors
        qt_big = bigpool.tile([128, E // 128, S], BF16, tag="qt", name="qt_big")
        kt_big = bigpool.tile([128, E // 128, NKC * 128], BF16, tag="kt", name="kt")
        v_big = bigpool.tile([128, NKC, E], BF16, tag="v", name="v")
        xkt_big = bigpool.tile(
            [128, D // 128, NKC * 128], BF16, tag="xkt", name="xkt"
        )

        for _rep in range(reps):
            # PSUM pools: proj phases use trpool+prpool (4 banks); the
            # attention block uses its own 8 banks after these close.
            with (
                tc.tile_pool(name="trp", bufs=2, space="PSUM") as trpool,
                tc.tile_pool(name="prp", bufs=2, space="PSUM") as prpool,
            ):
                # ============ Q phase: project + transpose ==============
                def xq_load(u):
                    xl2 = xq2pool.tile([128, 2, D], BF16, tag="xl2", name=f"xq{u}")
                    src = xq_d[u * 256 : (u + 1) * 256, :].rearrange(
                        "(c p) d -> p c d", p=128
                    )
                    if u == 0:
                        # split the first load so tile 0 lands (and the PE
                        # starts transposing) half a DMA earlier
                        nc.sync.dma_start(xl2[:, 0], src[:, 0])
                        nc.sync.dma_start(xl2[:, 1], src[:, 1])
                    else:
                        nc.sync.dma_start(xl2[:], src)
                    return xl2

                def xq_tr(xl2, i):
                    xt = xtpool.tile(
                        [128, D // 128, 128], BF16, tag="xqt", name=f"xqt{i}"
                    )
                    for dc in range(D // 128):
                        pst = trpool.tile([128, 128], BF16, tag="tr", name="trq")
                        nc.tensor.transpose(
                            pst[:], xl2[:, i % 2, dc * 128 : (dc + 1) * 128], ident
                        )
                        nc.vector.tensor_copy(xt[:, dc], pst[:])
                    return xt

                xls = {0: xq_load(0)}
                xts = {0: xq_tr(xls[0], 0), 1: xq_tr(xls[0], 1)}
                wq_sb = wload(wq_d, "q")
                qrow2 = None
                for i in range(NQT):
                    u, c = divmod(i, 2)
                    xt = xts.pop(i)
                    if c == 0:
                        qrow2 = qrowpool.tile(
                            [128, 2, E], BF16, tag="qrow", name=f"q{u}"
                        )
                    for eh in range(2):
                        pr = prpool.tile([128, 512], F32, tag="pr", name="prq")
                        for dc in range(D // 128):
                            nc.tensor.matmul(
                                pr[:],
                                xt[:, dc],
                                wq_sb[:, dc, eh * 512 : (eh + 1) * 512],
                                start=(dc == 0),
                                stop=(dc == D // 128 - 1),
                            )
                        if eh == 0:
                            nc.vector.tensor_copy(
                                qrow2[:, c, :512], pr[:]
                            )
                        else:
                            nc.scalar.copy(qrow2[:, c, 512:], pr[:])
                    if c == 1:
                        nc.gpsimd.dma_start(
                            qo_d[u * 256 : (u + 1) * 256, :].rearrange(
                                "(c p) e -> p c e", p=128
                            ),
                            qrow2[:],
                        )
                    # prefetch xq transpose i+2 between Q matmuls and Q^T
                    # transposes: covers the psum->qrow drain latency on PE
                    # and keeps the xl DMA a pair ahead of its use
                    if i + 2 < NQT:
                        if (i + 2) % 2 == 0:
                            xls[u + 1] = xq_load(u + 1)
                            xls.pop(u - 1, None)
                        xts[i + 2] = xq_tr(xls[(i + 2) // 2], i + 2)
                    for ec in range(E // 128):
                        pst = trpool.tile([128, 128], BF16, tag="tr", name="trq2")
                        nc.tensor.transpose(
                            pst[:], qrow2[:, c, ec * 128 : (ec + 1) * 128], ident
                        )
                        nc.vector.tensor_copy(
                            qt_big[:, ec, i * 128 : (i + 1) * 128], pst[:]
                        )

                # ============ KV phase ==================================
                wk_sb = wload(wk_d, "k")
                wv_sb = wload(wv_d, "v")

                def kv_load(g):
                    """Load xk/xv chunk pair g (chunks 2g, 2g+1)."""
                    xk2 = kv2pool.tile([128, 2, D], BF16, tag="xk2", name=f"xk{g}")
                    nc.sync.dma_start(
                        xk2[:],
                        xk_d[g * 256 : (g + 1) * 256, :].rearrange(
                            "(c p) d -> p c d", p=128
                        ),
                    )
                    xv2 = kv2pool.tile([128, 2, D], BF16, tag="xv2", name=f"xv{g}")
                    nc.sync.dma_start(
                        xv2[:],
                        xv_d[g * 256 : (g + 1) * 256, :].rearrange(
                            "(c p) d -> p c d", p=128
                        ),
                    )
                    return xk2, xv2

                def kv_tr(kvl, j):
                    """Transpose xk chunk j into xkt_big; return xv^T chunk."""
                    xk2, xv2 = kvl
                    for dc in range(D // 128):
                        pst = trpool.tile([128, 128], BF16, tag="tr", name="trk")
                        nc.tensor.transpose(
                            pst[:], xk2[:, j % 2, dc * 128 : (dc + 1) * 128], ident
                        )
                        nc.vector.tensor_copy(
                            xkt_big[:, dc, j * 128 : (j + 1) * 128], pst[:]
                        )
                    xvt = xtpool.tile(
                        [128, D // 128, 128], BF16, tag="xvt", name=f"xvt{j}"
                    )
                    for dc in range(D // 128):
                        pst = trpool.tile([128, 128], BF16, tag="tr", name="trv")
                        nc.tensor.transpose(
                            pst[:], xv2[:, j % 2, dc * 128 : (dc + 1) * 128], ident
                        )
                        nc.vector.tensor_copy(xvt[:, dc], pst[:])
                    return xvt

                def v_proj(j, xvt):
                    for eh in range(2):
                        pr = prpool.tile([128, 512], F32, tag="pr", name="prv")
                        for dc in range(D // 128):
                            nc.tensor.matmul(
                                pr[:],
                                xvt[:, dc],
                                wv_sb[:, dc, eh * 512 : (eh + 1) * 512],
                                start=(dc == 0),
                                stop=(dc == D // 128 - 1),
                            )
                        if eh == 0:
                            nc.vector.tensor_copy(v_big[:, j, :512], pr[:])
                        else:
                            nc.scalar.copy(v_big[:, j, 512:], pr[:])

                kvls = {0: kv_load(0)}
                xvts = {}
                for h in range(2):
                    for j in range(4 * h, 4 * h + 4):
                        if j % 2 == 0 and j // 2 + 1 < 4:
                            kvls[j // 2 + 1] = kv_load(j // 2 + 1)
                            kvls.pop(j // 2 - 1, None)
                        xvts[j] = kv_tr(kvls[j // 2], j)
                        if j - 1 in xvts:
                            v_proj(j - 1, xvts.pop(j - 1))
                    for ec in range(E // 128):
                        pr = prpool.tile([128, 512], F32, tag="pr", name="prk")
                        for dc in range(D // 128):
                            nc.tensor.matmul(
                                pr[:],
                                wk_sb[:, dc, ec * 128 : (ec + 1) * 128],
                                xkt_big[:, dc, h * 512 : (h + 1) * 512],
                                start=(dc == 0),
                                stop=(dc == D // 128 - 1),
                            )
                        if ec % 2 == 0:
                            nc.vector.tensor_copy(
                                kt_big[:, ec, h * 512 : (h + 1) * 512], pr[:]
                            )
                        else:
                            nc.scalar.copy(
                                kt_big[:, ec, h * 512 : (h + 1) * 512], pr[:]
                            )
                v_proj(7, xvts.pop(7))

            # ============ attention =====================================
            with (
                tc.tile_pool(name="spsp", bufs=2, space="PSUM") as spspool,
                tc.tile_pool(name="psap", bufs=4, space="PSUM") as psapool,
                tc.tile_pool(name="pslp", bufs=2, space="PSUM") as pslpool,
            ):
                # descending order: the big tiles' stores overlap the
                # remaining compute and the kernel ends on the 1-chunk tile
                for t in range(NAT - 1, -1, -1):
                    a_ps = [
                        psapool.tile([128, 512], F32, tag="aps", name=f"a{t}_{q}")
                        for q in range(4)
                    ]
                    l_ps = [
                        pslpool.tile([128, 2], F32, tag="lps", name=f"l{t}_{st}")
                        for st in range(2)
                    ]

                    def st_mm(j, t=t):
                        sps = spspool.tile([128, 256], F32, tag="sps", name="sps")
                        for ec in range(E // 128):
                            nc.tensor.matmul(
                                sps[:],
                                kt_big[:, ec, j * 128 : (j + 1) * 128],
                                qt_big[:, ec, t * 256 : (t + 1) * 256],
                                start=(ec == 0),
                                stop=(ec == E // 128 - 1),
                            )
                        pt = ptpool.tile([128, 256], BF16, tag="pt", name="pt")
                        nc.scalar.activation(
                            pt[:], sps[:], mybir.ActivationFunctionType.Exp,
                            scale=SCALE,
                        )
                        if j == t:
                            nc.vector.tensor_mul(pt[:], pt[:], maskt)
                        return pt

                    pts = {0: st_mm(0)}
                    for j in range(t + 1):
                        if j + 1 <= t:
                            pts[j + 1] = st_mm(j + 1)
                        pt = pts.pop(j)
                        first, last = j == 0, j == t
                        for st in range(2):
                            nc.tensor.matmul(
                                l_ps[st][:],
                                pt[:, st * 128 : (st + 1) * 128],
                                ones,
                                start=first,
                                stop=last,
                            )
                            for eh in range(2):
                                nc.tensor.matmul(
                                    a_ps[2 * st + eh][:],
                                    pt[:, st * 128 : (st + 1) * 128],
                                    v_big[:, j, eh * 512 : (eh + 1) * 512],
                                    start=first,
                                    stop=last,
                                )
                    at2 = outpool.tile([128, 2, E], BF16, tag="at", name=f"at{t}")
                    for st in range(2):
                        if st == 0:
                            nc.vector.tensor_copy(at2[:, st, :512], a_ps[2 * st][:])
                            nc.scalar.copy(at2[:, st, 512:], a_ps[2 * st + 1][:])
                        else:
                            nc.scalar.copy(at2[:, st, :512], a_ps[2 * st][:])
                            nc.vector.tensor_copy(
                                at2[:, st, 512:], a_ps[2 * st + 1][:]
                            )
                        nc.vector.tensor_copy(
                            ls_big[:, t, st : st + 1], l_ps[st][:, 0:1]
                        )
                    if t == 0:
                        # l store goes out before the final a store
                        nc.sync.dma_start(
                            lo_d.rearrange("t s p -> p t s"), ls_big[:]
                        )
                    # alternate store queues to halve per-queue wire time
                    dq = nc.gpsimd if t % 2 == 0 else nc.sync
                    dq.dma_start(
                        ao_d[t * 256 : (t + 1) * 256, :].rearrange(
                            "(s p) e -> p s e", p=128
                        ),
                        at2[:],
                    )


def _mask(s: int) -> np.ndarray:
    kr = np.arange(128)[:, None]
    qr = np.arange(256)[None, :]
    return (s * 128 + kr <= qr).astype(NPB)


def _consts(s: int) -> np.ndarray:
    out = np.zeros((128, NCON), NPB)
    out[:, :128] = np.eye(128, dtype=np.float32).astype(NPB)
    out[:, 128:130] = 1.0
    out[:, 130:386] = _mask(s)
    return out


def make_core_inputs(xq_b, xk_b, xv_b, wq, wk, wv, s):
    """Per-core input dict. xq_b/xk_b/xv_b are the batch's [S, D] arrays."""
    return {
        "xq": np.ascontiguousarray(xq_b.astype(NPB)),
        "xk": np.ascontiguousarray(
            xk_b.reshape(16, 128, D)[s::2].reshape(NKC * 128, D).astype(NPB)
        ),
        "xv": np.ascontiguousarray(
            xv_b.reshape(16, 128, D)[s::2].reshape(NKC * 128, D).astype(NPB)
        ),
        "wq": wq,
        "wk": wk,
        "wv": wv,
        "consts": _consts(s),
    }


_NC_CACHE = {}


def kernel(inputs_for_keys, inputs_for_values, inputs_for_queries, WK, WV, WQ):
    if "nc" not in _NC_CACHE:
        _NC_CACHE["nc"] = build_nc(1)
    nc = _NC_CACHE["nc"]

    xk = np.asarray(inputs_for_keys, np.float32)
    xv = np.asarray(inputs_for_values, np.float32)
    xq = np.asarray(inputs_for_queries, np.float32)
    wk = np.ascontiguousarray(np.asarray(WK, np.float32).astype(NPB))
    wv = np.ascontiguousarray(np.asarray(WV, np.float32).astype(NPB))
    wq = np.ascontiguousarray(np.asarray(WQ, np.float32).astype(NPB))

    in_maps = []
    for c in range(NCORES):
        b, s = c // 2, c % 2
        in_maps.append(make_core_inputs(xq[b], xk[b], xv[b], wq, wk, wv, s))
    res = run_bass_kernel_spmd(nc, in_maps, list(range(NCORES)))

    q_full = np.empty((B, S, E), np.float32)
    a_full = np.empty((B, S, E), np.float32)
    for b in range(B):
        r0 = res.results[2 * b]
        r1 = res.results[2 * b + 1]
        a = r0["a_out"].astype(np.float32) + r1["a_out"].astype(np.float32)
        l = (r0["l_out"] + r1["l_out"]).reshape(S)
        a_full[b] = a / l[:, None]
        q_full[b] = r0["q_out"].astype(np.float32)
    return q_full, a_full
